# revision 31
# baseline (speedup 1.0000x reference)
"""Trainium2 Bass kernel for nn_CortexReasoner (masked-update attention with
Iron RoPE + relative Fourier bias).

Sharding: one attention head per NeuronCore (n_head == n_cores == 8), both
batches on every core; the output projection is redistributed with an
All-to-All so each core finalizes a disjoint 512-row slice of the output.

Precision ladder (tolerance 2e-2): the QKV projection and the whole
score/softmax/AV path run in fp8e4 with DoubleRow matmuls (2 contraction
rows per PE pass); softmax exp carries a -3 shift so e^(s-3) fits fp8
range and cancels in the normalization.  The rope rotation and the output
projection run in bf16; PSUM accumulation is always fp32.
"""

import math
import os
import sys

import numpy as np
import ml_dtypes

for _p in ("/opt/trn_rl_repo",):
    if _p not in sys.path and os.path.isdir(_p):
        sys.path.append(_p)

import concourse.bass as bass
import concourse.mybir as mybir
import concourse.tile as tile
from concourse.bass_utils import run_bass_kernel_spmd
F32 = mybir.dt.float32
F32R = mybir.dt.float32r
BF16 = mybir.dt.bfloat16
F16 = mybir.dt.float16
FP8 = mybir.dt.float8e4
AF = mybir.ActivationFunctionType
DR = mybir.MatmulPerfMode.DoubleRow
NP_BF16 = np.dtype(ml_dtypes.bfloat16)
NP_FP8 = np.dtype(ml_dtypes.float8_e4m3)

B, T, D = 2, 2048, 1024
H = 8
HD = 128          # head dim
N_CORES = 8
BT = B * T        # 4096
ROWS = BT // N_CORES   # 512 output rows per core
NCH = 8           # t-chunks of 512 across B*T
CT2 = D // 256    # 4 double-row contraction tiles for the QKV projection
CT = D // 128     # 8 contraction tiles for the output projection
KT = T // 128     # 16 key tiles per batch
KT2 = KT // 2     # 8 double-row key-tile pairs
QC = T // 512     # 4 query chunks per batch

ESHIFT = -3.0     # exp(s + ESHIFT): fp8e4 max 240, keeps worst-case in range
WARM0 = 48        # startup p-state warm matmuls ([128,128])
WARM3 = 148       # pre/during-A2A warm matmuls ([128,512])


def _build_nc():
    nc = bass.Bass()

    xp = nc.dram_tensor("xp", [CT2, NCH, 128, 2, 512], FP8, kind="ExternalInput")
    wq = nc.dram_tensor("wq", [128, CT2, 2, HD], FP8, kind="ExternalInput")
    wk = nc.dram_tensor("wk", [128, CT2, 2, HD], FP8, kind="ExternalInput")
    wv = nc.dram_tensor("wv", [128, CT2, 2, HD], FP8, kind="ExternalInput")
    bq = nc.dram_tensor("bq", [HD, 2], F32, kind="ExternalInput")  # col1: halves swapped
    bk = nc.dram_tensor("bk", [HD, 2], F32, kind="ExternalInput")
    bv = nc.dram_tensor("bv", [HD, 1], F32, kind="ExternalInput")
    At = nc.dram_tensor("At", [B, 128, T], BF16, kind="ExternalInput")    # [cos;cos]
    Bt = nc.dram_tensor("Bt", [B, 128, T], BF16, kind="ExternalInput")    # [-sin;sin]
    fk8 = nc.dram_tensor("fk8", [B, 128, T], FP8, kind="ExternalInput")   # zero-padded 64:128
    fq8 = nc.dram_tensor("fq8", [B, 128, T], FP8, kind="ExternalInput")
    identity = nc.dram_tensor("identity", [128, 128], BF16, kind="ExternalInput")
    ones2 = nc.dram_tensor("ones2", [128, 2, 128], FP8, kind="ExternalInput")
    wo = nc.dram_tensor("wo", [128, CT2, 2, D], FP8, kind="ExternalInput")
    maskc = nc.dram_tensor("maskc", [ROWS, 1], F32, kind="ExternalInput")
    in1m = nc.dram_tensor("in1m", [ROWS, D], BF16, kind="ExternalInput")

    out = nc.dram_tensor("out", [ROWS, D], BF16, kind="ExternalOutput")

    with tile.TileContext(nc) as tc, \
         nc.allow_low_precision(reason="fp8/bf16 matmul pipeline"):
        with tc.tile_pool(name="persist", bufs=1) as pp, \
             tc.tile_pool(name="consts", bufs=1) as cp, \
             tc.tile_pool(name="dram", bufs=1, space="DRAM") as dp:

            # [128, 2, T]: subtile 0 = rope-rotated q/k, subtile 1 = fourier
            # features (zero-padded on partitions 64:128)
            qrot = [pp.tile([128, 2, T], FP8, tag=f"qrot{b}", name=f"qrot{b}") for b in range(B)]
            krot = [pp.tile([128, 2, T], FP8, tag=f"krot{b}", name=f"krot{b}") for b in range(B)]
            vnat = [pp.tile([128, KT, 128], FP8, tag=f"vnat{b}", name=f"vnat{b}") for b in range(B)]

            t1s = cp.tile([128, 2, 128], FP8)  # all-ones DoubleRow lhsT (M=128)
            ident = cp.tile([128, 128], BF16)
            eshift = cp.tile([128, 1], F32)
            nc.any.memset(eshift[:], ESHIFT)
            warmsrc = cp.tile([128, 128], BF16)
            nc.any.memset(warmsrc[:], 0.5)
            nc.sync.dma_start(out=ident[:], in_=identity[:])
            nc.sync.dma_start(out=t1s[:], in_=ones2[:])

            # Startup warm loop: ramp the PE clock while the first weight/x
            # DMAs stream in, so phase-1 matmuls run at full p-state.
            with tc.tile_pool(name="warm0", bufs=1, space="PSUM") as pw0:
                pwt = pw0.tile([128, 128], F32, name="pwt")
                for wj in range(WARM0):
                    nc.tensor.matmul(pwt[:], warmsrc[:], warmsrc[:],
                                     start=(wj == 0), stop=(wj == WARM0 - 1))

            # ---------------- Phase 1: QKV projection + RoPE + V transpose
            with tc.tile_pool(name="ph1", bufs=1) as p1, \
                 tc.tile_pool(name="ph1x", bufs=8) as p1x, \
                 tc.tile_pool(name="ph1s", bufs=6) as p1s, \
                 tc.tile_pool(name="ph1t", bufs=9) as p1t, \
                 tc.tile_pool(name="ps1", bufs=5, space="PSUM") as ps1, \
                 tc.tile_pool(name="ps1c", bufs=2, space="PSUM") as ps1c:

                wqt = p1.tile([128, CT2, 2, HD], FP8)
                wkt = p1.tile([128, CT2, 2, HD], FP8)
                wvt = p1.tile([128, CT2, 2, HD], FP8)
                # per-ct2 weight loads so the first matmul fires as soon as
                # its own slice lands, not after all 3 full weight tensors
                for ct2 in range(CT2):
                    nc.sync.dma_start(out=wqt[:, ct2], in_=wq[:, ct2])
                    nc.sync.dma_start(out=wkt[:, ct2], in_=wk[:, ct2])
                    nc.sync.dma_start(out=wvt[:, ct2], in_=wv[:, ct2])
                tbq = p1.tile([128, 2], F32)
                tbk = p1.tile([128, 2], F32)
                tbv = p1.tile([128, 1], F32)
                nc.sync.dma_start(out=tbq[:], in_=bq[:])
                nc.sync.dma_start(out=tbk[:], in_=bk[:])
                nc.sync.dma_start(out=tbv[:], in_=bv[:])
                tAt = [p1.tile([128, T], BF16, tag=f"At{b}", name=f"tAt{b}") for b in range(B)]
                tBt = [p1.tile([128, T], BF16, tag=f"Bt{b}", name=f"tBt{b}") for b in range(B)]

                for ch in range(NCH):
                    b = ch // (NCH // B)
                    tch_b = slice((ch % 4) * 512, (ch % 4 + 1) * 512)
                    xts = []
                    for ct2 in range(CT2):
                        xt = p1x.tile([128, 2, 512], FP8, tag="xt")
                        nc.sync.dma_start(out=xt[:], in_=xp[ct2, ch])
                        xts.append(xt)
                    if ch == 0:
                        nc.sync.dma_start(out=tAt[0][:], in_=At[0])
                        nc.sync.dma_start(out=tBt[0][:], in_=Bt[0])
                    elif ch == 1:
                        nc.sync.dma_start(out=tAt[1][:], in_=At[1])
                        nc.sync.dma_start(out=tBt[1][:], in_=Bt[1])
                    elif ch == 2:
                        # fourier features go into subtile 1 (pre-padded)
                        for bb in range(B):
                            nc.sync.dma_start(out=qrot[bb][:, 1, :], in_=fq8[bb])
                            nc.sync.dma_start(out=krot[bb][:, 1, :], in_=fk8[bb])
                    pq = ps1.tile([128, 512], F32, tag="pqkv")
                    pk = ps1.tile([128, 512], F32, tag="pqkv")
                    pv = ps1.tile([128, 512], F32, tag="pqkv")
                    for ct2 in range(CT2):
                        st, sp = (ct2 == 0), (ct2 == CT2 - 1)
                        nc.tensor.matmul(pq[:], wqt[:, ct2], xts[ct2][:], start=st, stop=sp, perf_mode=DR)
                        nc.tensor.matmul(pk[:], wkt[:, ct2], xts[ct2][:], start=st, stop=sp, perf_mode=DR)
                        nc.tensor.matmul(pv[:], wvt[:, ct2], xts[ct2][:], start=st, stop=sp, perf_mode=DR)

                    # v: bias then transpose 4x 128x128 into vnat
                    sv = p1s.tile([128, 512], BF16, tag="sv")
                    nc.scalar.activation(sv[:], pv[:], AF.Identity, bias=tbv[:])
                    for j in range(4):
                        ptr = ps1c.tile([128, 128], BF16, tag="ptr")
                        nc.tensor.transpose(ptr[:], sv[:, j * 128:(j + 1) * 128], ident[:])
                        g = (ch % 4) * 4 + j
                        nc.vector.tensor_copy(vnat[b][:, g, :], ptr[:])

                    # q/k: add bias, rope-rotate into qrot/krot subtile 0.
                    # The half-swap runs as a cheap SBUF->SBUF DMA (partition
                    # shift is free for DMA), so no PE permutation matmul or
                    # Act copy is needed and all multiplies are full-width.
                    for qi, (psrc, tb, dstl) in enumerate(((pq, tbq, qrot), (pk, tbk, krot))):
                        dst = dstl[b]
                        sqk = p1s.tile([128, 512], BF16, tag="sqk")
                        nc.scalar.activation(sqk[:], psrc[:], AF.Identity, bias=tb[:, 0:1])
                        sw = p1t.tile([128, 512], BF16, tag="ropeS")
                        # issued from the Act queue: Act just produced sqk, so
                        # this adds no cross-engine stall, and keeps the sync
                        # queue free for the x-chunk streaming loads
                        nc.scalar.dma_start(out=sw[0:64, :], in_=sqk[64:128, :])
                        nc.scalar.dma_start(out=sw[64:128, :], in_=sqk[0:64, :])
                        ta = p1t.tile([128, 512], BF16, tag="ropeA")
                        tbm = p1t.tile([128, 512], BF16, tag="ropeB")
                        nc.vector.tensor_mul(ta[:], sqk[:], tAt[b][:, tch_b])
                        if qi == 0:
                            nc.gpsimd.tensor_mul(tbm[:], sw[:], tBt[b][:, tch_b])
                            nc.gpsimd.tensor_add(dst[:, 0, tch_b], ta[:], tbm[:])
                        else:
                            nc.vector.tensor_mul(tbm[:], sw[:], tBt[b][:, tch_b])
                            nc.vector.tensor_add(dst[:, 0, tch_b], ta[:], tbm[:])

            # phase-3 prefetches: queue after everything phase 2 needs;
            # they drain during phase 2
            two = cp.tile([128, CT2, 2, D], FP8)
            nc.sync.dma_start(out=two[:], in_=wo[:])
            tmask = cp.tile([128, 4], F32)
            nc.sync.dma_start(out=tmask[:], in_=maskc.rearrange("(tt p) one -> p (tt one)", p=128))
            tin1 = cp.tile([128, 4 * D], BF16)
            for tt in range(4):
                nc.sync.dma_start(out=tin1[:, tt * D:(tt + 1) * D],
                                  in_=in1m[tt * 128:(tt + 1) * 128, :])

            # ---------------- Phase 2: attention
            a2a_in = dp.tile([N_CORES, 128, 512], FP8)
            a2a_out = dp.tile([N_CORES, 128, 512], FP8)
            # tiny warm-up collective: pays the CC dispatch/bootstrap cost
            # during phase 1/2 so the real AllToAll starts faster
            warm_cc_in = dp.tile([N_CORES, 1, 16], FP8)
            warm_cc_out = dp.tile([N_CORES, 1, 16], FP8)
            nc.gpsimd.collective_compute(
                "AllToAll", mybir.AluOpType.bypass,
                ins=[warm_cc_in.opt()], outs=[warm_cc_out.opt()],
                replica_groups=[list(range(N_CORES))],
            )
            with tc.tile_pool(name="ph2e", bufs=5) as p2e, \
                 tc.tile_pool(name="ph2r", bufs=2) as p2r, \
                 tc.tile_pool(name="ps2", bufs=2, space="PSUM") as ps2, \
                 tc.tile_pool(name="ps2y", bufs=2, space="PSUM") as ps2y, \
                 tc.tile_pool(name="ps2s", bufs=2, space="PSUM") as ps2s:

                for u in range(B):
                    for qc in range(QC):
                        qs = slice(qc * 512, (qc + 1) * 512)
                        py = ps2y.tile([128, 512], F32, tag="py")
                        # rowsum via all-ones stationary with redundant M=128:
                        # every partition of psm holds the rowsum, so the
                        # normalize is a plain elementwise multiply (no PE
                        # broadcast matmul on the critical path).
                        psm = ps2s.tile([128, 512], F32, tag="psm")
                        lag = []
                        for kt2 in range(KT2):
                            psc = ps2.tile([128, 1024], F32, tag="psc")
                            for hh in range(2):
                                kt = 2 * kt2 + hh
                                ks = slice(kt * 128, (kt + 1) * 128)
                                nc.tensor.matmul(psc[:, hh * 512:(hh + 1) * 512],
                                                 krot[u][:, :, ks], qrot[u][:, :, qs],
                                                 start=True, stop=True, perf_mode=DR)
                            se = p2e.tile([128, 2, 512], FP8, tag="exp")
                            nc.scalar.activation(se[:], psc[:], AF.Exp, bias=eshift[:])
                            lag.append((kt2, se))
                            if kt2 >= 1:
                                pk2, pse = lag.pop(0)
                                nc.tensor.matmul(py[:], vnat[u][:, 2 * pk2:2 * pk2 + 2, :], pse[:],
                                                 start=(pk2 == 0), stop=(pk2 == KT2 - 1), perf_mode=DR)
                                nc.tensor.matmul(psm[:], t1s[:], pse[:],
                                                 start=(pk2 == 0), stop=(pk2 == KT2 - 1), perf_mode=DR)
                        for pk2, pse in lag:
                            nc.tensor.matmul(py[:], vnat[u][:, 2 * pk2:2 * pk2 + 2, :], pse[:],
                                             start=(pk2 == 0), stop=(pk2 == KT2 - 1), perf_mode=DR)
                            nc.tensor.matmul(psm[:], t1s[:], pse[:],
                                             start=(pk2 == 0), stop=(pk2 == KT2 - 1), perf_mode=DR)
                        rr = p2r.tile([128, 512], F32, tag="rr")
                        nc.vector.reciprocal(rr[:], psm[:])
                        ynrm = p2r.tile([128, 512], FP8, tag="ynrm")
                        nc.vector.tensor_mul(ynrm[:], py[:], rr[:])
                        nc.sync.dma_start(out=a2a_in[u * QC + qc], in_=ynrm[:])

            # ---------------- Phase 3: A2A redistribute + output projection
            with tc.tile_pool(name="ph3", bufs=1) as p3, \
                 tc.tile_pool(name="ph3s", bufs=4) as p3s, \
                 tc.tile_pool(name="ps3", bufs=3, space="PSUM") as ps3:

                pwarm = ps3.tile([128, 512], F32, tag="pwarm", name="pwarm")
                for wj in range(WARM3):
                    nc.tensor.matmul(pwarm[:], two[:, 0, 0, 0:128], two[:, 0, 0, 0:512],
                                     start=(wj == 0), stop=(wj == WARM3 - 1))
                nc.gpsimd.collective_compute(
                    "AllToAll", mybir.AluOpType.bypass,
                    ins=[a2a_in.opt()], outs=[a2a_out.opt()],
                    replica_groups=[list(range(N_CORES))],
                )
                yab = p3.tile([128, N_CORES, 512], FP8)
                for dt in range(N_CORES):
                    nc.sync.dma_start(out=yab[:, dt, :], in_=a2a_out[dt])
                for tt in range(4):
                    for nch in range(2):
                        po = ps3.tile([128, 512], F32, tag="po")
                        for c2 in range(CT2):
                            nc.tensor.matmul(po[:], yab[:, 2 * c2:2 * c2 + 2, tt * 128:(tt + 1) * 128],
                                             two[:, c2, :, nch * 512:(nch + 1) * 512],
                                             start=(c2 == 0), stop=(c2 == CT2 - 1), perf_mode=DR)
                        so = p3s.tile([128, 512], BF16, tag="so")
                        nc.vector.scalar_tensor_tensor(
                            out=so[:], in0=po[:], scalar=tmask[:, tt:tt + 1],
                            in1=tin1[:, tt * D + nch * 512: tt * D + (nch + 1) * 512],
                            op0=mybir.AluOpType.mult, op1=mybir.AluOpType.add)
                        nc.sync.dma_start(out=out[tt * 128:(tt + 1) * 128, nch * 512:(nch + 1) * 512], in_=so[:])

    _split_multi_waits(nc)
    return nc


def _split_multi_waits(nc):
    """This walrus build encodes at most one sync-wait per instruction; hoist
    extras onto preceding NoOps.  For the kernel-tail drain (many DMA-queue
    waits, followed by an all-engine barrier) spread the NoOps across all
    engines so the waits poll in parallel; elsewhere keep them on the same
    engine to preserve ordering semantics."""
    engs = [mybir.EngineType.SP, mybir.EngineType.Activation, mybir.EngineType.DVE,
            mybir.EngineType.PE, mybir.EngineType.Pool]
    for f in nc.m.functions:
        for bb in f.blocks:
            new_insts = []
            for inst in bb.instructions:
                si = inst.sync_info
                if si is not None and si.on_wait and len(si.on_wait) > 1:
                    waits = list(si.on_wait)
                    distribute = (type(inst).__name__ == "InstDrain"
                                  and len(waits) > 3)
                    for j, w in enumerate(waits[:-1]):
                        eng = engs[j % len(engs)] if distribute else inst.engine
                        new_insts.append(mybir.InstNoOp(
                            name=f"{inst.name}_wsplit{j}", ins=[], outs=[],
                            engine=eng,
                            sync_info=mybir.SyncInfo(on_wait=[w], on_update=[])))
                    si.on_wait = [waits[-1]]
                new_insts.append(inst)
            bb.instructions = new_insts


ROPE_M = 64
FB_M = 32


def _prep_inputs(x, coords, update_mask, Wqkv, bqkv, Wo, bo, W_rope, W_fb,
                 beta_cos, beta_sin):
    """Per-core input maps (host-side layout + tiny trig tables)."""
    f32 = np.float32
    x = np.asarray(x, f32)
    coords = np.asarray(coords, f32)
    update_mask = np.asarray(update_mask)
    Wqkv = np.asarray(Wqkv, f32)
    bqkv = np.asarray(bqkv, f32)
    Wo = np.ascontiguousarray(np.asarray(Wo, f32))
    bo = np.asarray(bo, f32)
    W_rope = np.asarray(W_rope, f32)
    W_fb = np.asarray(W_fb, f32)
    beta_cos = np.asarray(beta_cos, f32)
    beta_sin = np.asarray(beta_sin, f32)

    xf = x.reshape(BT, D)
    xT = xf.T  # [D, BT]
    # DoubleRow layout: xp[ct2, ch, p, j, c] = xT[ct2*256 + j*128 + p, ch*512 + c]
    xp = np.ascontiguousarray(
        xT.reshape(CT2, 2, 128, NCH, 512).transpose(0, 3, 2, 1, 4).astype(NP_FP8))

    # split-half channel order: evens then odds
    perm = np.concatenate([np.arange(0, HD, 2), np.arange(1, HD, 2)])
    # balance the 1/sqrt(hd) between q and k so both stay in fp8 sweet spot
    sc_half = f32(HD ** -0.25)

    # rope tables per batch: theta[m, t]; A=[cos;cos], B=[-sin;sin]
    At = np.empty((B, 128, T), NP_BF16)
    Bts = np.empty((B, 128, T), NP_BF16)
    fkT = np.zeros((B, 128, T), NP_FP8)
    fqT = np.zeros((B, 128, T), NP_FP8)
    for b in range(B):
        c1 = coords[b, :, 0].astype(np.float64)
        th = (W_rope[:, 0:1].astype(np.float64) * c1[None, :])
        cth = np.cos(th).astype(f32)
        sth = np.sin(th).astype(f32)
        At[b] = np.concatenate([cth, cth], axis=0).astype(NP_BF16)
        Bts[b] = np.concatenate([-sth, sth], axis=0).astype(NP_BF16)
        S = (W_fb[:, 0:1].astype(np.float64) * c1[None, :])
        cS = np.cos(S).astype(f32)
        sS = np.sin(S).astype(f32)
        fkT[b, :64] = np.concatenate([cS, sS], axis=0).astype(NP_FP8)
        fqT[b, :64] = np.concatenate(
            [cS * beta_cos[:, None] + sS * beta_sin[:, None],
             sS * beta_cos[:, None] - cS * beta_sin[:, None]], axis=0).astype(NP_FP8)

    ones2 = np.ones((128, 2, 128), NP_FP8)
    ident_np = np.eye(128, dtype=NP_BF16)

    mask_f = update_mask.reshape(BT).astype(f32)

    # Wo in DoubleRow rhs layout: wo_dr[p, c2, j, :] = Wo[(2*c2+j)*128 + p, :]
    wo_dr = np.ascontiguousarray(
        Wo.astype(NP_FP8).reshape(CT2, 2, 128, D).transpose(2, 0, 1, 3))

    def dr_weight(wcol, scale):
        # [D, HD] scaled -> [128, CT2, 2, HD] DoubleRow stationary layout
        w = (wcol * scale).astype(NP_FP8)
        return np.ascontiguousarray(w.reshape(CT2, 2, 128, HD).transpose(2, 0, 1, 3))

    in_maps = []
    for c in range(N_CORES):
        h = c
        wq_h = dr_weight(Wqkv[:, h * HD:(h + 1) * HD][:, perm], sc_half)
        wk_h = dr_weight(Wqkv[:, D + h * HD:D + (h + 1) * HD][:, perm], sc_half)
        wv_h = dr_weight(Wqkv[:, 2 * D + h * HD:2 * D + (h + 1) * HD], f32(1.0))
        def bias2(bcol):
            bs = np.concatenate([bcol[64:], bcol[:64]])
            return np.stack([bcol, bs], axis=1)
        bq_h = bias2(bqkv[h * HD:(h + 1) * HD][perm] * sc_half)
        bk_h = bias2(bqkv[D + h * HD:D + (h + 1) * HD][perm] * sc_half)
        bv_h = bqkv[2 * D + h * HD:2 * D + (h + 1) * HD].reshape(HD, 1)
        rows = slice(c * ROWS, (c + 1) * ROWS)
        mrows = mask_f[rows].reshape(ROWS, 1)
        in1 = mrows * bo[None, :] + (1.0 - mrows) * xf[rows]
        in_maps.append(dict(
            xp=xp, wq=wq_h, wk=wk_h, wv=wv_h,
            bq=np.ascontiguousarray(bq_h.astype(f32)),
            bk=np.ascontiguousarray(bk_h.astype(f32)),
            bv=np.ascontiguousarray(bv_h.astype(f32)),
            At=At, Bt=Bts, fk8=fkT, fq8=fqT,
            ones2=ones2, wo=wo_dr, identity=ident_np,
            maskc=np.ascontiguousarray(mrows),
            in1m=np.ascontiguousarray(in1.astype(NP_BF16)),
        ))
    return in_maps


_NC_CACHE = None


def _get_nc():
    global _NC_CACHE
    if _NC_CACHE is None:
        _NC_CACHE = _build_nc()
    return _NC_CACHE


def run(trace=False, **inputs):
    nc = _get_nc()
    in_maps = _prep_inputs(**inputs)
    res = run_bass_kernel_spmd(nc, in_maps, core_ids=list(range(N_CORES)),
                               trace=trace)
    outs = [np.asarray(res.results[c]["out"]).astype(np.float32) for c in range(N_CORES)]
    full = np.concatenate(outs, axis=0).reshape(B, T, D)
    return full, res


def kernel(**inputs) -> np.ndarray:
    full, _ = run(trace=False, **inputs)
    return full



# revision 33
# speedup vs baseline: 1.0411x; 1.0411x over previous
"""Trainium2 Bass kernel for nn_CortexReasoner (masked-update attention with
Iron RoPE + relative Fourier bias).

Sharding: one attention head per NeuronCore (n_head == n_cores == 8), both
batches on every core; the output projection is redistributed with an
All-to-All so each core finalizes a disjoint 512-row slice of the output.

Precision ladder (tolerance 2e-2): the QKV projection and the whole
score/softmax/AV path run in fp8e4 with DoubleRow matmuls (2 contraction
rows per PE pass); softmax exp carries a -3 shift so e^(s-3) fits fp8
range and cancels in the normalization.  The rope rotation and the output
projection run in bf16; PSUM accumulation is always fp32.
"""

import math
import os
import sys

import numpy as np
import ml_dtypes

for _p in ("/opt/trn_rl_repo",):
    if _p not in sys.path and os.path.isdir(_p):
        sys.path.append(_p)

import concourse.bass as bass
import concourse.mybir as mybir
import concourse.tile as tile
from concourse.bass_utils import run_bass_kernel_spmd
F32 = mybir.dt.float32
F32R = mybir.dt.float32r
BF16 = mybir.dt.bfloat16
F16 = mybir.dt.float16
FP8 = mybir.dt.float8e4
AF = mybir.ActivationFunctionType
DR = mybir.MatmulPerfMode.DoubleRow
NP_BF16 = np.dtype(ml_dtypes.bfloat16)
NP_FP8 = np.dtype(ml_dtypes.float8_e4m3)

B, T, D = 2, 2048, 1024
H = 8
HD = 128          # head dim
N_CORES = 8
BT = B * T        # 4096
ROWS = BT // N_CORES   # 512 output rows per core
NCH = 8           # t-chunks of 512 across B*T
CT2 = D // 256    # 4 double-row contraction tiles for the QKV projection
CT = D // 128     # 8 contraction tiles for the output projection
KT = T // 128     # 16 key tiles per batch
KT2 = KT // 2     # 8 double-row key-tile pairs
QC = T // 512     # 4 query chunks per batch

ESHIFT = -3.0     # exp(s + ESHIFT): fp8e4 max 240, keeps worst-case in range
WARM0 = 48        # startup p-state warm matmuls ([128,128])
WARM3 = 148       # pre/during-A2A warm matmuls ([128,512])


def _build_nc():
    nc = bass.Bass()

    xp = nc.dram_tensor("xp", [CT2, NCH, 128, 2, 512], FP8, kind="ExternalInput")
    wq = nc.dram_tensor("wq", [128, CT2, 2, HD], FP8, kind="ExternalInput")
    wk = nc.dram_tensor("wk", [128, CT2, 2, HD], FP8, kind="ExternalInput")
    wv = nc.dram_tensor("wv", [128, CT2, 2, HD], FP8, kind="ExternalInput")
    bq = nc.dram_tensor("bq", [HD, 2], F32, kind="ExternalInput")  # col1: halves swapped
    bk = nc.dram_tensor("bk", [HD, 2], F32, kind="ExternalInput")
    bv = nc.dram_tensor("bv", [HD, 1], F32, kind="ExternalInput")
    At = nc.dram_tensor("At", [B, 128, T], BF16, kind="ExternalInput")    # [cos;cos]
    Bt = nc.dram_tensor("Bt", [B, 128, T], BF16, kind="ExternalInput")    # [-sin;sin]
    fk8 = nc.dram_tensor("fk8", [B, 128, T], FP8, kind="ExternalInput")   # zero-padded 64:128
    fq8 = nc.dram_tensor("fq8", [B, 128, T], FP8, kind="ExternalInput")
    identity = nc.dram_tensor("identity", [128, 128], BF16, kind="ExternalInput")
    ones2 = nc.dram_tensor("ones2", [128, 2, 128], FP8, kind="ExternalInput")
    wo = nc.dram_tensor("wo", [128, CT2, 2, D], FP8, kind="ExternalInput")
    maskc = nc.dram_tensor("maskc", [ROWS, 1], F32, kind="ExternalInput")
    in1m = nc.dram_tensor("in1m", [ROWS, D], BF16, kind="ExternalInput")

    out = nc.dram_tensor("out", [ROWS, D], BF16, kind="ExternalOutput")

    with tile.TileContext(nc) as tc, \
         nc.allow_low_precision(reason="fp8/bf16 matmul pipeline"):
        with tc.tile_pool(name="persist", bufs=1) as pp, \
             tc.tile_pool(name="consts", bufs=1) as cp, \
             tc.tile_pool(name="dram", bufs=1, space="DRAM") as dp:

            # [128, 2, T]: subtile 0 = rope-rotated q/k, subtile 1 = fourier
            # features (zero-padded on partitions 64:128)
            qrot = [pp.tile([128, 2, T], FP8, tag=f"qrot{b}", name=f"qrot{b}") for b in range(B)]
            krot = [pp.tile([128, 2, T], FP8, tag=f"krot{b}", name=f"krot{b}") for b in range(B)]
            vnat = [pp.tile([128, KT, 128], FP8, tag=f"vnat{b}", name=f"vnat{b}") for b in range(B)]

            t1s = cp.tile([128, 2, 128], FP8)  # all-ones DoubleRow lhsT (M=128)
            ident = cp.tile([128, 128], BF16)
            eshift = cp.tile([128, 1], F32)
            nc.any.memset(eshift[:], ESHIFT)
            warmsrc = cp.tile([128, 128], BF16)
            nc.any.memset(warmsrc[:], 0.5)
            nc.sync.dma_start(out=ident[:], in_=identity[:])
            nc.sync.dma_start(out=t1s[:], in_=ones2[:])

            # Startup warm loop: ramp the PE clock while the first weight/x
            # DMAs stream in, so phase-1 matmuls run at full p-state.
            with tc.tile_pool(name="warm0", bufs=1, space="PSUM") as pw0:
                pwt = pw0.tile([128, 128], F32, name="pwt")
                for wj in range(WARM0):
                    nc.tensor.matmul(pwt[:], warmsrc[:], warmsrc[:],
                                     start=(wj == 0), stop=(wj == WARM0 - 1))

            # ---------------- Phase 1: QKV projection + RoPE + V transpose
            with tc.tile_pool(name="ph1", bufs=1) as p1, \
                 tc.tile_pool(name="ph1x", bufs=8) as p1x, \
                 tc.tile_pool(name="ph1s", bufs=6) as p1s, \
                 tc.tile_pool(name="ph1t", bufs=9) as p1t, \
                 tc.tile_pool(name="ps1", bufs=5, space="PSUM") as ps1, \
                 tc.tile_pool(name="ps1c", bufs=2, space="PSUM") as ps1c:

                wqt = p1.tile([128, CT2, 2, HD], FP8)
                wkt = p1.tile([128, CT2, 2, HD], FP8)
                wvt = p1.tile([128, CT2, 2, HD], FP8)
                nc.sync.dma_start(out=wqt[:], in_=wq[:])
                nc.sync.dma_start(out=wkt[:], in_=wk[:])
                nc.sync.dma_start(out=wvt[:], in_=wv[:])
                tbq = p1.tile([128, 2], F32)
                tbk = p1.tile([128, 2], F32)
                tbv = p1.tile([128, 1], F32)
                nc.sync.dma_start(out=tbq[:], in_=bq[:])
                nc.sync.dma_start(out=tbk[:], in_=bk[:])
                nc.sync.dma_start(out=tbv[:], in_=bv[:])
                tAt = [p1.tile([128, T], BF16, tag=f"At{b}", name=f"tAt{b}") for b in range(B)]
                tBt = [p1.tile([128, T], BF16, tag=f"Bt{b}", name=f"tBt{b}") for b in range(B)]

                for ch in range(NCH):
                    b = ch // (NCH // B)
                    tch_b = slice((ch % 4) * 512, (ch % 4 + 1) * 512)
                    xts = []
                    for ct2 in range(CT2):
                        xt = p1x.tile([128, 2, 512], FP8, tag="xt")
                        nc.sync.dma_start(out=xt[:], in_=xp[ct2, ch])
                        xts.append(xt)
                    if ch == 0:
                        nc.sync.dma_start(out=tAt[0][:], in_=At[0])
                        nc.sync.dma_start(out=tBt[0][:], in_=Bt[0])
                    elif ch == 1:
                        nc.sync.dma_start(out=tAt[1][:], in_=At[1])
                        nc.sync.dma_start(out=tBt[1][:], in_=Bt[1])
                    elif ch == 2:
                        # fourier features go into subtile 1 (pre-padded)
                        for bb in range(B):
                            nc.sync.dma_start(out=qrot[bb][:, 1, :], in_=fq8[bb])
                            nc.sync.dma_start(out=krot[bb][:, 1, :], in_=fk8[bb])
                    pq = ps1.tile([128, 512], F32, tag="pqkv")
                    pk = ps1.tile([128, 512], F32, tag="pqkv")
                    pv = ps1.tile([128, 512], F32, tag="pqkv")
                    for ct2 in range(CT2):
                        st, sp = (ct2 == 0), (ct2 == CT2 - 1)
                        nc.tensor.matmul(pq[:], wqt[:, ct2], xts[ct2][:], start=st, stop=sp, perf_mode=DR)
                        nc.tensor.matmul(pk[:], wkt[:, ct2], xts[ct2][:], start=st, stop=sp, perf_mode=DR)
                        nc.tensor.matmul(pv[:], wvt[:, ct2], xts[ct2][:], start=st, stop=sp, perf_mode=DR)

                    # v: bias then transpose 4x 128x128 into vnat
                    sv = p1s.tile([128, 512], BF16, tag="sv")
                    nc.scalar.activation(sv[:], pv[:], AF.Identity, bias=tbv[:])
                    for j in range(4):
                        ptr = ps1c.tile([128, 128], BF16, tag="ptr")
                        nc.tensor.transpose(ptr[:], sv[:, j * 128:(j + 1) * 128], ident[:])
                        g = (ch % 4) * 4 + j
                        nc.scalar.activation(vnat[b][:, g, :], ptr[:], AF.Copy)

                    # q/k: add bias, rope-rotate into qrot/krot subtile 0.
                    # The half-swap runs as a cheap SBUF->SBUF DMA (partition
                    # shift is free for DMA), so no PE permutation matmul or
                    # Act copy is needed and all multiplies are full-width.
                    for qi, (psrc, tb, dstl) in enumerate(((pq, tbq, qrot), (pk, tbk, krot))):
                        dst = dstl[b]
                        sqk = p1s.tile([128, 512], BF16, tag="sqk")
                        nc.scalar.activation(sqk[:], psrc[:], AF.Identity, bias=tb[:, 0:1])
                        sw = p1t.tile([128, 512], BF16, tag="ropeS")
                        # issued from the Act queue: Act just produced sqk, so
                        # this adds no cross-engine stall, and keeps the sync
                        # queue free for the x-chunk streaming loads
                        nc.scalar.dma_start(out=sw[0:64, :], in_=sqk[64:128, :])
                        nc.scalar.dma_start(out=sw[64:128, :], in_=sqk[0:64, :])
                        ta = p1t.tile([128, 512], BF16, tag="ropeA")
                        tbm = p1t.tile([128, 512], BF16, tag="ropeB")
                        nc.vector.tensor_mul(ta[:], sqk[:], tAt[b][:, tch_b])
                        if qi == 0:
                            nc.gpsimd.tensor_mul(tbm[:], sw[:], tBt[b][:, tch_b])
                            nc.gpsimd.tensor_add(dst[:, 0, tch_b], ta[:], tbm[:])
                        else:
                            nc.vector.tensor_mul(tbm[:], sw[:], tBt[b][:, tch_b])
                            nc.vector.tensor_add(dst[:, 0, tch_b], ta[:], tbm[:])

            # phase-3 prefetches: queue after everything phase 2 needs;
            # they drain during phase 2
            two = cp.tile([128, CT2, 2, D], FP8)
            nc.sync.dma_start(out=two[:], in_=wo[:])
            tmask = cp.tile([128, 4], F32)
            nc.sync.dma_start(out=tmask[:], in_=maskc.rearrange("(tt p) one -> p (tt one)", p=128))
            tin1 = cp.tile([128, 4 * D], BF16)
            for tt in range(4):
                nc.sync.dma_start(out=tin1[:, tt * D:(tt + 1) * D],
                                  in_=in1m[tt * 128:(tt + 1) * 128, :])

            # ---------------- Phase 2: attention
            a2a_in = dp.tile([N_CORES, 128, 512], FP8)
            a2a_out = dp.tile([N_CORES, 128, 512], FP8)
            # tiny warm-up collective: pays the CC dispatch/bootstrap cost
            # during phase 1/2 so the real AllToAll starts faster
            warm_cc_in = dp.tile([N_CORES, 1, 16], FP8)
            warm_cc_out = dp.tile([N_CORES, 1, 16], FP8)
            nc.gpsimd.collective_compute(
                "AllToAll", mybir.AluOpType.bypass,
                ins=[warm_cc_in.opt()], outs=[warm_cc_out.opt()],
                replica_groups=[list(range(N_CORES))],
            )
            with tc.tile_pool(name="ph2e", bufs=5) as p2e, \
                 tc.tile_pool(name="ph2r", bufs=2) as p2r, \
                 tc.tile_pool(name="ps2", bufs=2, space="PSUM") as ps2, \
                 tc.tile_pool(name="ps2y", bufs=2, space="PSUM") as ps2y, \
                 tc.tile_pool(name="ps2s", bufs=2, space="PSUM") as ps2s:

                for u in range(B):
                    for qc in range(QC):
                        qs = slice(qc * 512, (qc + 1) * 512)
                        py = ps2y.tile([128, 512], F32, tag="py")
                        # rowsum via all-ones stationary with redundant M=128:
                        # every partition of psm holds the rowsum, so the
                        # normalize is a plain elementwise multiply (no PE
                        # broadcast matmul on the critical path).
                        psm = ps2s.tile([128, 512], F32, tag="psm")
                        lag = []
                        for kt2 in range(KT2):
                            psc = ps2.tile([128, 1024], F32, tag="psc")
                            for hh in range(2):
                                kt = 2 * kt2 + hh
                                ks = slice(kt * 128, (kt + 1) * 128)
                                nc.tensor.matmul(psc[:, hh * 512:(hh + 1) * 512],
                                                 krot[u][:, :, ks], qrot[u][:, :, qs],
                                                 start=True, stop=True, perf_mode=DR)
                            se = p2e.tile([128, 2, 512], FP8, tag="exp")
                            nc.scalar.activation(se[:], psc[:], AF.Exp, bias=eshift[:])
                            lag.append((kt2, se))
                            if kt2 >= 1:
                                pk2, pse = lag.pop(0)
                                nc.tensor.matmul(py[:], vnat[u][:, 2 * pk2:2 * pk2 + 2, :], pse[:],
                                                 start=(pk2 == 0), stop=(pk2 == KT2 - 1), perf_mode=DR)
                                nc.tensor.matmul(psm[:], t1s[:], pse[:],
                                                 start=(pk2 == 0), stop=(pk2 == KT2 - 1), perf_mode=DR)
                        for pk2, pse in lag:
                            nc.tensor.matmul(py[:], vnat[u][:, 2 * pk2:2 * pk2 + 2, :], pse[:],
                                             start=(pk2 == 0), stop=(pk2 == KT2 - 1), perf_mode=DR)
                            nc.tensor.matmul(psm[:], t1s[:], pse[:],
                                             start=(pk2 == 0), stop=(pk2 == KT2 - 1), perf_mode=DR)
                        rr = p2r.tile([128, 512], F32, tag="rr")
                        nc.vector.reciprocal(rr[:], psm[:])
                        ynrm = p2r.tile([128, 512], FP8, tag="ynrm")
                        nc.vector.tensor_mul(ynrm[:], py[:], rr[:])
                        nc.sync.dma_start(out=a2a_in[u * QC + qc], in_=ynrm[:])

            # ---------------- Phase 3: A2A redistribute + output projection
            with tc.tile_pool(name="ph3", bufs=1) as p3, \
                 tc.tile_pool(name="ph3s", bufs=4) as p3s, \
                 tc.tile_pool(name="ps3", bufs=3, space="PSUM") as ps3:

                pwarm = ps3.tile([128, 512], F32, tag="pwarm", name="pwarm")
                for wj in range(WARM3):
                    nc.tensor.matmul(pwarm[:], two[:, 0, 0, 0:128], two[:, 0, 0, 0:512],
                                     start=(wj == 0), stop=(wj == WARM3 - 1))
                nc.gpsimd.collective_compute(
                    "AllToAll", mybir.AluOpType.bypass,
                    ins=[a2a_in.opt()], outs=[a2a_out.opt()],
                    replica_groups=[list(range(N_CORES))],
                )
                yab = p3.tile([128, N_CORES, 512], FP8)
                for dt in range(N_CORES):
                    nc.sync.dma_start(out=yab[:, dt, :], in_=a2a_out[dt])
                for tt in range(4):
                    for nch in range(2):
                        po = ps3.tile([128, 512], F32, tag="po")
                        for c2 in range(CT2):
                            nc.tensor.matmul(po[:], yab[:, 2 * c2:2 * c2 + 2, tt * 128:(tt + 1) * 128],
                                             two[:, c2, :, nch * 512:(nch + 1) * 512],
                                             start=(c2 == 0), stop=(c2 == CT2 - 1), perf_mode=DR)
                        so = p3s.tile([128, 512], BF16, tag="so")
                        nc.vector.scalar_tensor_tensor(
                            out=so[:], in0=po[:], scalar=tmask[:, tt:tt + 1],
                            in1=tin1[:, tt * D + nch * 512: tt * D + (nch + 1) * 512],
                            op0=mybir.AluOpType.mult, op1=mybir.AluOpType.add)
                        nc.sync.dma_start(out=out[tt * 128:(tt + 1) * 128, nch * 512:(nch + 1) * 512], in_=so[:])

    _split_multi_waits(nc)
    return nc


def _split_multi_waits(nc):
    """This walrus build encodes at most one sync-wait per instruction; hoist
    extras onto preceding NoOps.  For the kernel-tail drain (many DMA-queue
    waits, followed by an all-engine barrier) spread the NoOps across all
    engines so the waits poll in parallel; elsewhere keep them on the same
    engine to preserve ordering semantics."""
    engs = [mybir.EngineType.SP, mybir.EngineType.Activation, mybir.EngineType.DVE,
            mybir.EngineType.PE, mybir.EngineType.Pool]
    for f in nc.m.functions:
        for bb in f.blocks:
            new_insts = []
            for inst in bb.instructions:
                si = inst.sync_info
                if si is not None and si.on_wait and len(si.on_wait) > 1:
                    waits = list(si.on_wait)
                    distribute = (type(inst).__name__ == "InstDrain"
                                  and len(waits) > 3)
                    for j, w in enumerate(waits[:-1]):
                        eng = engs[j % len(engs)] if distribute else inst.engine
                        new_insts.append(mybir.InstNoOp(
                            name=f"{inst.name}_wsplit{j}", ins=[], outs=[],
                            engine=eng,
                            sync_info=mybir.SyncInfo(on_wait=[w], on_update=[])))
                    si.on_wait = [waits[-1]]
                new_insts.append(inst)
            bb.instructions = new_insts


ROPE_M = 64
FB_M = 32


def _prep_inputs(x, coords, update_mask, Wqkv, bqkv, Wo, bo, W_rope, W_fb,
                 beta_cos, beta_sin):
    """Per-core input maps (host-side layout + tiny trig tables)."""
    f32 = np.float32
    x = np.asarray(x, f32)
    coords = np.asarray(coords, f32)
    update_mask = np.asarray(update_mask)
    Wqkv = np.asarray(Wqkv, f32)
    bqkv = np.asarray(bqkv, f32)
    Wo = np.ascontiguousarray(np.asarray(Wo, f32))
    bo = np.asarray(bo, f32)
    W_rope = np.asarray(W_rope, f32)
    W_fb = np.asarray(W_fb, f32)
    beta_cos = np.asarray(beta_cos, f32)
    beta_sin = np.asarray(beta_sin, f32)

    xf = x.reshape(BT, D)
    xT = xf.T  # [D, BT]
    # DoubleRow layout: xp[ct2, ch, p, j, c] = xT[ct2*256 + j*128 + p, ch*512 + c]
    xp = np.ascontiguousarray(
        xT.reshape(CT2, 2, 128, NCH, 512).transpose(0, 3, 2, 1, 4).astype(NP_FP8))

    # split-half channel order: evens then odds
    perm = np.concatenate([np.arange(0, HD, 2), np.arange(1, HD, 2)])
    # balance the 1/sqrt(hd) between q and k so both stay in fp8 sweet spot
    sc_half = f32(HD ** -0.25)

    # rope tables per batch: theta[m, t]; A=[cos;cos], B=[-sin;sin]
    At = np.empty((B, 128, T), NP_BF16)
    Bts = np.empty((B, 128, T), NP_BF16)
    fkT = np.zeros((B, 128, T), NP_FP8)
    fqT = np.zeros((B, 128, T), NP_FP8)
    for b in range(B):
        c1 = coords[b, :, 0].astype(np.float64)
        th = (W_rope[:, 0:1].astype(np.float64) * c1[None, :])
        cth = np.cos(th).astype(f32)
        sth = np.sin(th).astype(f32)
        At[b] = np.concatenate([cth, cth], axis=0).astype(NP_BF16)
        Bts[b] = np.concatenate([-sth, sth], axis=0).astype(NP_BF16)
        S = (W_fb[:, 0:1].astype(np.float64) * c1[None, :])
        cS = np.cos(S).astype(f32)
        sS = np.sin(S).astype(f32)
        fkT[b, :64] = np.concatenate([cS, sS], axis=0).astype(NP_FP8)
        fqT[b, :64] = np.concatenate(
            [cS * beta_cos[:, None] + sS * beta_sin[:, None],
             sS * beta_cos[:, None] - cS * beta_sin[:, None]], axis=0).astype(NP_FP8)

    ones2 = np.ones((128, 2, 128), NP_FP8)
    ident_np = np.eye(128, dtype=NP_BF16)

    mask_f = update_mask.reshape(BT).astype(f32)

    # Wo in DoubleRow rhs layout: wo_dr[p, c2, j, :] = Wo[(2*c2+j)*128 + p, :]
    wo_dr = np.ascontiguousarray(
        Wo.astype(NP_FP8).reshape(CT2, 2, 128, D).transpose(2, 0, 1, 3))

    def dr_weight(wcol, scale):
        # [D, HD] scaled -> [128, CT2, 2, HD] DoubleRow stationary layout
        w = (wcol * scale).astype(NP_FP8)
        return np.ascontiguousarray(w.reshape(CT2, 2, 128, HD).transpose(2, 0, 1, 3))

    in_maps = []
    for c in range(N_CORES):
        h = c
        wq_h = dr_weight(Wqkv[:, h * HD:(h + 1) * HD][:, perm], sc_half)
        wk_h = dr_weight(Wqkv[:, D + h * HD:D + (h + 1) * HD][:, perm], sc_half)
        wv_h = dr_weight(Wqkv[:, 2 * D + h * HD:2 * D + (h + 1) * HD], f32(1.0))
        def bias2(bcol):
            bs = np.concatenate([bcol[64:], bcol[:64]])
            return np.stack([bcol, bs], axis=1)
        bq_h = bias2(bqkv[h * HD:(h + 1) * HD][perm] * sc_half)
        bk_h = bias2(bqkv[D + h * HD:D + (h + 1) * HD][perm] * sc_half)
        bv_h = bqkv[2 * D + h * HD:2 * D + (h + 1) * HD].reshape(HD, 1)
        rows = slice(c * ROWS, (c + 1) * ROWS)
        mrows = mask_f[rows].reshape(ROWS, 1)
        in1 = mrows * bo[None, :] + (1.0 - mrows) * xf[rows]
        in_maps.append(dict(
            xp=xp, wq=wq_h, wk=wk_h, wv=wv_h,
            bq=np.ascontiguousarray(bq_h.astype(f32)),
            bk=np.ascontiguousarray(bk_h.astype(f32)),
            bv=np.ascontiguousarray(bv_h.astype(f32)),
            At=At, Bt=Bts, fk8=fkT, fq8=fqT,
            ones2=ones2, wo=wo_dr, identity=ident_np,
            maskc=np.ascontiguousarray(mrows),
            in1m=np.ascontiguousarray(in1.astype(NP_BF16)),
        ))
    return in_maps


_NC_CACHE = None


def _get_nc():
    global _NC_CACHE
    if _NC_CACHE is None:
        _NC_CACHE = _build_nc()
    return _NC_CACHE


def run(trace=False, **inputs):
    nc = _get_nc()
    in_maps = _prep_inputs(**inputs)
    res = run_bass_kernel_spmd(nc, in_maps, core_ids=list(range(N_CORES)),
                               trace=trace)
    outs = [np.asarray(res.results[c]["out"]).astype(np.float32) for c in range(N_CORES)]
    full = np.concatenate(outs, axis=0).reshape(B, T, D)
    return full, res


def kernel(**inputs) -> np.ndarray:
    full, _ = run(trace=False, **inputs)
    return full

